# revision 15
# baseline (speedup 1.0000x reference)
"""Distributed Trainium2 Bass kernel for the Asymm-3D sparse submanifold conv
ResContextBlock (gnn_message_passing).

Computation (reference):
    s = BN(lrelu(subm_conv(feats, W1)));  s = BN(lrelu(subm_conv(s, W1_2)))
    r = BN(lrelu(subm_conv(feats, W2))); r = BN(lrelu(subm_conv(r, W3)))
    out = r + s
with training-mode BN over all 200000 active voxels.

Sharding: voxels sorted spatially (batch,z-major), 8 shards x 25000 owned rows.
Each core works on E = owned + halo rows; host resolves all sparse-conv
neighbor indices and pre-gathers first-conv (128-ch) neighbor features.
Device: dense center matmuls + offset-segmented pair matmuls with
dma_scatter_add into HBM accumulators; dma_gather(transpose) for second-conv
neighbor values; BN stats via masked ones-matmul + AllReduce of [2,32] sums.
"""

import os
import sys

sys.path.insert(0, "/opt/trn_rl_repo")

import numpy as np
import ml_dtypes

BF16 = ml_dtypes.bfloat16

GRID = (480, 360, 32)
GZ, GY, GX = GRID
BATCH = 2
N_VOX = 200000
C_IN = 128
C_OUT = 16
EPS = 1e-5
NEG = 0.01
NCORES = 8
M_OWN = N_VOX // NCORES  # 25000
CHUNK = 512

# kernel (1,3,3): oz=0, oy,ox in {-1,0,1}   (W1, W3)
OFFS_A = [(0, dy, dx) for dy in (-1, 0, 1) for dx in (-1, 0, 1) if (dy, dx) != (0, 0)]
# kernel (3,1,3): oz,ox in {-1,0,1}, oy=0   (W2, W1_2)
OFFS_B = [(dz, 0, dx) for dz in (-1, 0, 1) for dx in (-1, 0, 1) if (dz, dx) != (0, 0)]

LAST_RESULT = None  # BassKernelResults of the last run (test.py reads exec_time_ns)


def _ceil(a, b):
    return -(-a // b) * b


def _wrap16(ids, dtype=np.int16):
    """idx j -> [j % 16, j // 16], replicated across the 8 gpsimd cores
    (128 partitions) as dma_gather/dma_scatter_add expect."""
    ids = np.asarray(ids)
    assert len(ids) % 16 == 0
    w = np.ascontiguousarray(ids.reshape(-1, 16).T.astype(dtype))
    return np.ascontiguousarray(np.tile(w, (8, 1)))


def _host_prep(feats, coords):
    """Partition + build all per-core staged arrays and compile-time sizes."""
    N = coords.shape[0]
    b = coords[:, 0].astype(np.int64)
    z = coords[:, 1].astype(np.int64)
    y = coords[:, 2].astype(np.int64)
    x = coords[:, 3].astype(np.int64)
    lin = ((b * GZ + z) * GY + y) * GX + x
    order = np.argsort(lin, kind="stable").astype(np.int64)

    grid = np.full(BATCH * GZ * GY * GX, -1, np.int32)
    grid[lin] = np.arange(N, dtype=np.int32)

    def neighbor(rows, off):
        oz, oy, ox = off
        zz = z[rows] + oz
        yy = y[rows] + oy
        xx = x[rows] + ox
        valid = (zz >= 0) & (zz < GZ) & (yy >= 0) & (yy < GY) & (xx >= 0) & (xx < GX)
        l2 = ((b[rows] * GZ + np.clip(zz, 0, GZ - 1)) * GY + np.clip(yy, 0, GY - 1)) * GX + np.clip(xx, 0, GX - 1)
        nb = grid[l2].copy()
        nb[~valid] = -1
        return nb

    cores = []
    for k in range(NCORES):
        own_g = order[k * M_OWN : (k + 1) * M_OWN]
        in_own = np.zeros(N, bool)
        in_own[own_g] = True
        halo = []
        for off in OFFS_B + OFFS_A:  # stage-2 neighbor universe (W1_2 and W3)
            nb = neighbor(own_g, off)
            nb = nb[nb >= 0]
            halo.append(nb[~in_own[nb]])
        halo_g = np.unique(np.concatenate(halo)) if halo else np.empty(0, np.int64)
        E_g = np.concatenate([own_g, np.sort(halo_g)]).astype(np.int64)
        NE = len(E_g)
        g2l = np.full(N, -1, np.int32)
        g2l[E_g] = np.arange(NE, dtype=np.int32)

        def stage1_pairs(offs):
            segs = []
            for off in offs:
                nb = neighbor(E_g, off)
                valid = nb >= 0
                segs.append((nb[valid].astype(np.int64), np.nonzero(valid)[0].astype(np.int32)))
            return segs

        def stage2_pairs(offs):
            segs = []
            for off in offs:
                nb = neighbor(own_g, off)
                valid = nb >= 0
                src_l = g2l[nb[valid]]
                assert (src_l >= 0).all()
                segs.append((src_l.astype(np.int32), np.nonzero(valid)[0].astype(np.int32)))
            return segs

        cores.append(
            dict(
                own_g=own_g,
                E_g=E_g,
                NE=NE,
                p1=stage1_pairs(OFFS_A),   # conv W1
                p2=stage1_pairs(OFFS_B),   # conv W2
                p3=stage2_pairs(OFFS_B),   # conv W1_2 (on s)
                p4=stage2_pairs(OFFS_A),   # conv W3  (on r)
            )
        )

    NEpad = _ceil(max(c["NE"] for c in cores) + 1, CHUNK)  # +1 for trash row
    assert NEpad <= 32767, NEpad
    seg_sizes = {}
    for key in ("p1", "p2", "p3", "p4"):
        seg_sizes[key] = [
            _ceil(max(len(c[key][i][0]) for c in cores), 128) for i in range(8)
        ]
    sizes = dict(
        NEpad=NEpad,
        n_chunks=NEpad // CHUNK,
        P1=sum(seg_sizes["p1"]),
        P2=sum(seg_sizes["p2"]),
        P3=sum(seg_sizes["p3"]),
        P4=sum(seg_sizes["p4"]),
        segs=seg_sizes,
    )

    trash = NEpad - 1

    in_maps = []
    for c in cores:
        m = {}
        ftx = np.zeros((NEpad, C_IN), np.float32)
        ftx[: c["NE"]] = feats[c["E_g"]]
        m["featsT"] = np.ascontiguousarray(ftx.T.astype(BF16))

        for key, Pn, nbname, dstname in (
            ("p1", sizes["P1"], "nb1T", "dst1"),
            ("p2", sizes["P2"], "nb2T", "dst2"),
        ):
            nb = np.zeros((Pn, C_IN), np.float32)
            dst = np.full(Pn, trash, np.int32)
            pos = 0
            for i in range(8):
                src_g, dst_l = c[key][i]
                nb[pos : pos + len(src_g)] = feats[src_g]
                dst[pos : pos + len(dst_l)] = dst_l
                pos += sizes["segs"][key][i]
            m[nbname] = np.ascontiguousarray(nb.T.astype(BF16))
            m[dstname] = _wrap16(dst)

        gsrc = np.full(sizes["P3"] + sizes["P4"], trash, np.int32)
        for key, base in (("p3", 0), ("p4", sizes["P3"])):
            pos = base
            for i in range(8):
                src_l, _ = c[key][i]
                gsrc[pos : pos + len(src_l)] = src_l
                pos += sizes["segs"][key][i]
        m["gsrc"] = _wrap16(gsrc)
        for key, Pn, dstname in (("p3", sizes["P3"], "dst3"), ("p4", sizes["P4"], "dst4")):
            dst = np.full(Pn, trash, np.int32)
            pos = 0
            for i in range(8):
                _, dst_l = c[key][i]
                dst[pos : pos + len(dst_l)] = dst_l
                pos += sizes["segs"][key][i]
            m[dstname] = _wrap16(dst)
        in_maps.append(m)

    return cores, sizes, in_maps


def _build_graph(sizes, W1, g0, b0, W1_2, g0_2, b0_2, W2, g1, b1, W3, g2, b2):
    import concourse.bass as bass
    import concourse.bacc as bacc
    import concourse.tile as tile
    from concourse import mybir

    f32 = mybir.dt.float32
    bf16 = mybir.dt.bfloat16
    i16 = mybir.dt.int16
    AF = mybir.ActivationFunctionType
    ALU = mybir.AluOpType
    AP = bass.AP

    NEpad = sizes["NEpad"]
    n_chunks = sizes["n_chunks"]
    OWN_CHUNKS = -(-M_OWN // CHUNK)                 # 49
    OWNpad = OWN_CHUNKS * CHUNK                     # 25088
    last_valid = M_OWN - (OWN_CHUNKS - 1) * CHUNK   # 424
    P1, P2, P3, P4 = sizes["P1"], sizes["P2"], sizes["P3"], sizes["P4"]
    P34 = P3 + P4

    nc = bacc.Bacc("TRN2", target_bir_lowering=False, debug=False, num_devices=NCORES)

    # ---------------- external I/O ----------------
    featsT = nc.dram_tensor("featsT", [C_IN, NEpad], bf16, kind="ExternalInput")
    nb1T = nc.dram_tensor("nb1T", [C_IN, P1], bf16, kind="ExternalInput")
    nb2T = nc.dram_tensor("nb2T", [C_IN, P2], bf16, kind="ExternalInput")
    dst1 = nc.dram_tensor("dst1", [128, P1 // 16], i16, kind="ExternalInput")
    dst2 = nc.dram_tensor("dst2", [128, P2 // 16], i16, kind="ExternalInput")
    gsrc = nc.dram_tensor("gsrc", [128, P34 // 16], i16, kind="ExternalInput")
    dst3 = nc.dram_tensor("dst3", [128, P3 // 16], i16, kind="ExternalInput")
    dst4 = nc.dram_tensor("dst4", [128, P4 // 16], i16, kind="ExternalInput")
    out_ext = nc.dram_tensor("out", [M_OWN, C_OUT], f32, kind="ExternalOutput")

    # ---------------- inline constants (identical on all cores) -------------
    def packA(W):
        return [W[0, dy + 1, dx + 1] for (dz, dy, dx) in OFFS_A]

    def packB(W):
        return [W[dz + 1, 0, dx + 1] for (dz, dy, dx) in OFFS_B]

    Wc1_np = np.concatenate([W1[0, 1, 1], W2[1, 0, 1]], axis=1)  # [128, 32]
    W1o_np = np.stack(packA(W1), axis=1)    # [128, 8, 16]
    W2o_np = np.stack(packB(W2), axis=1)    # [128, 8, 16]
    Wc2_np = np.zeros((32, 32), np.float32)
    Wc2_np[0:16, 0:16] = W1_2[1, 0, 1]
    Wc2_np[16:32, 16:32] = W3[0, 1, 1]
    W12o_np = np.concatenate(
        [np.stack(packB(W1_2), axis=1), np.zeros((16, 8, 16), np.float32)], axis=0
    )  # [32, 8, 16]: rows 16:32 zero (kill r-channels)
    W3o_np = np.concatenate(
        [np.zeros((16, 8, 16), np.float32), np.stack(packA(W3), axis=1)], axis=0
    )  # [32, 8, 16]: rows 0:16 zero (kill s-channels)

    Wc1_d = nc.inline_tensor(Wc1_np.astype(BF16), "Wc1")
    W1o_d = nc.inline_tensor(np.ascontiguousarray(W1o_np.astype(BF16)), "W1o")
    W2o_d = nc.inline_tensor(np.ascontiguousarray(W2o_np.astype(BF16)), "W2o")
    Wc2_d = nc.inline_tensor(Wc2_np.astype(BF16), "Wc2")
    W12o_d = nc.inline_tensor(np.ascontiguousarray(W12o_np.astype(BF16)), "W12o")
    W3o_d = nc.inline_tensor(np.ascontiguousarray(W3o_np.astype(BF16)), "W3o")

    gb1_np = np.concatenate([g0, g1, b0, b1]).reshape(1, 64)
    gb2_np = np.concatenate([g0_2, g2, b0_2, b2]).reshape(1, 64)
    gb1_d = nc.inline_tensor(gb1_np.astype(np.float32), "gb1")
    gb2_d = nc.inline_tensor(gb2_np.astype(np.float32), "gb2")

    ident_d = nc.inline_tensor(np.eye(128, dtype=BF16), "ident")
    ones_d = nc.inline_tensor(np.ones((128, 1), BF16), "ones128")
    mask_np = np.zeros((128, 1), np.float32)
    mask_np[: max(0, last_valid - 384), 0] = 1.0  # block-3 partial rows of last owned chunk
    maskL_d = nc.inline_tensor(mask_np.astype(BF16), "maskL")

    ccin1 = nc.dram_tensor("ccin1", [1, 64], f32)
    ccout1 = nc.dram_tensor("ccout1", [1, 64], f32)
    ccin2 = nc.dram_tensor("ccin2", [1, 64], f32)
    ccout2 = nc.dram_tensor("ccout2", [1, 64], f32)

    acc1 = nc.dram_tensor("acc1", [NEpad, 64], f32)      # stage-1 pre-act (s|r|pad)
    s1r1n = nc.dram_tensor("s1r1n", [NEpad, 128], bf16)  # normalized rows (gather src)
    acc2 = nc.dram_tensor("acc2", [OWNpad, 64], f32)     # stage-2 pre-act

    def dram3(t, row0, rowstride, nblk, inner):
        """AP over dram tensor t: dims [128 rows-in-block, nblk blocks, inner],
        element (p, b, ch) -> row row0 + b*128 + p, col ch."""
        return AP(
            t,
            row0 * rowstride,
            [[rowstride, 128], [128 * rowstride, nblk], [1, inner]],
        )

    def bcast4(ap_, nblk=4):
        """[128, W] AP -> [128, (x nblk), W] stride-0 broadcast AP."""
        dims = [list(d) for d in ap_.ap]
        assert len(dims) == 2
        return AP(ap_.tensor, ap_.offset, [dims[0], [0, nblk], dims[1]])

    with tile.TileContext(nc) as tc:
        nb1_sb = nc.alloc_sbuf_tensor("nb1_sb", [C_IN, P1], bf16)
        nb2_sb = nc.alloc_sbuf_tensor("nb2_sb", [C_IN, P2], bf16)
        held1 = nc.alloc_sbuf_tensor("held1", [128, 4 * n_chunks, 32], bf16)
        held2 = nc.alloc_sbuf_tensor("held2", [128, 4 * OWN_CHUNKS, 32], bf16)
        s1r1T = nc.alloc_sbuf_tensor("s1r1T", [32, OWNpad], bf16)
        Wc1_sb = nc.alloc_sbuf_tensor("Wc1_sb", [C_IN, 32], bf16)
        W1o_sb = nc.alloc_sbuf_tensor("W1o_sb", [C_IN, 8, 16], bf16)
        W2o_sb = nc.alloc_sbuf_tensor("W2o_sb", [C_IN, 8, 16], bf16)
        Wc2_sb = nc.alloc_sbuf_tensor("Wc2_sb", [32, 32], bf16)
        W12o_sb = nc.alloc_sbuf_tensor("W12o_sb", [32, 8, 16], bf16)
        W3o_sb = nc.alloc_sbuf_tensor("W3o_sb", [32, 8, 16], bf16)
        ident_sb = nc.alloc_sbuf_tensor("ident_sb", [128, 128], bf16)
        ones_sb = nc.alloc_sbuf_tensor("ones_sb", [128, 1], bf16)
        maskL_sb = nc.alloc_sbuf_tensor("maskL_sb", [128, 1], bf16)
        gb1_sb = nc.alloc_sbuf_tensor("gb1_sb", [1, 64], f32)
        gb2_sb = nc.alloc_sbuf_tensor("gb2_sb", [1, 64], f32)
        dst1_sb = nc.alloc_sbuf_tensor("dst1_sb", [128, P1 // 16], i16)
        dst2_sb = nc.alloc_sbuf_tensor("dst2_sb", [128, P2 // 16], i16)
        gsrc_sb = nc.alloc_sbuf_tensor("gsrc_sb", [128, P34 // 16], i16)
        dst3_sb = nc.alloc_sbuf_tensor("dst3_sb", [128, P3 // 16], i16)
        dst4_sb = nc.alloc_sbuf_tensor("dst4_sb", [128, P4 // 16], i16)
        contrib1 = nc.alloc_sbuf_tensor("contrib1", [128, P1 // 128, 16], f32)
        contrib2 = nc.alloc_sbuf_tensor("contrib2", [128, P2 // 128, 16], f32)
        contrib3 = nc.alloc_sbuf_tensor("contrib3", [128, P3 // 128, 16], f32)
        contrib4 = nc.alloc_sbuf_tensor("contrib4", [128, P4 // 128, 16], f32)
        gath = nc.alloc_sbuf_tensor("gath", [128, 1, P34], bf16)
        wa = [nc.alloc_sbuf_tensor(f"wa{i}", [128, 4, 64], f32) for i in range(2)]
        wn = [nc.alloc_sbuf_tensor(f"wn{i}", [128, 4, 128], bf16) for i in range(2)]
        stat1x = nc.alloc_sbuf_tensor("stat1x", [1, 128], f32)
        stat1q = nc.alloc_sbuf_tensor("stat1q", [1, 128], f32)
        stat2x = nc.alloc_sbuf_tensor("stat2x", [1, 128], f32)
        stat2q = nc.alloc_sbuf_tensor("stat2q", [1, 128], f32)
        ab1_sb = nc.alloc_sbuf_tensor("ab1_sb", [128, 64], f32)
        ab2_sb = nc.alloc_sbuf_tensor("ab2_sb", [128, 64], f32)
        scratch = nc.alloc_sbuf_tensor("scratch", [1, 64], f32)
        B2sum = nc.alloc_sbuf_tensor("B2sum", [128, 16], f32)
        zb = nc.alloc_sbuf_tensor("zb", [128, 1], f32)

        # ---- constants / metadata loads ----
        for sb, d in (
            (Wc1_sb, Wc1_d), (W1o_sb, W1o_d), (W2o_sb, W2o_d), (Wc2_sb, Wc2_d),
            (W12o_sb, W12o_d), (W3o_sb, W3o_d), (ident_sb, ident_d),
            (ones_sb, ones_d), (maskL_sb, maskL_d), (gb1_sb, gb1_d),
            (gb2_sb, gb2_d), (dst1_sb, dst1), (dst2_sb, dst2),
            (gsrc_sb, gsrc), (dst3_sb, dst3), (dst4_sb, dst4),
        ):
            nc.gpsimd.dma_start(sb[:], d[:])
        nc.sync.dma_start(nb1_sb[:], nb1T[:])
        nc.sync.dma_start(nb2_sb[:], nb2T[:])
        nc.vector.memset(zb[:], 0.0)
        for w in wa:
            nc.vector.memset(w[:], 0.0)
        for w in wn:
            nc.vector.memset(w[:], 0.0)
        for s_ in (stat1x, stat1q, stat2x, stat2q):
            nc.vector.memset(s_[:], 0.0)

        # ================= stage 1: centers =================
        with (
            tc.tile_pool(name="ftp", bufs=3) as ftp,
            tc.tile_pool(name="ps1", bufs=2, space=bass.MemorySpace.PSUM) as ps1,
        ):
            for c in range(n_chunks):
                ft = ftp.tile([C_IN, CHUNK], bf16)
                nc.sync.dma_start(ft[:], featsT[:, c * CHUNK : (c + 1) * CHUNK])
                ps = ps1.tile([128, 4, 32], f32)
                for bk in range(4):
                    nc.tensor.matmul(
                        ps[:, bk, :], ft[:, bk * 128 : (bk + 1) * 128], Wc1_sb[:]
                    )
                w = wa[c % 2]
                nc.scalar.copy(w[:, :, 0:32], ps[:])
                nc.scalar.dma_start(dram3(acc1, c * CHUNK, 64, 4, 64), w[:])

        # ================= stage 1: sparse taps =================
        with tc.tile_pool(name="ps2", bufs=2, space=bass.MemorySpace.PSUM) as ps2:
            for (nbsb, wo_sb, ctb, key) in (
                (nb1_sb, W1o_sb, contrib1, "p1"),
                (nb2_sb, W2o_sb, contrib2, "p2"),
            ):
                pos = 0
                for o in range(8):
                    seg = sizes["segs"][key][o]
                    for s0 in range(pos, pos + seg, 128):
                        psc = ps2.tile([128, 16], f32)
                        nc.tensor.matmul(
                            psc[:], nbsb[:, s0 : s0 + 128], wo_sb[:, o, :]
                        )
                        nc.vector.tensor_copy(ctb[:, s0 // 128, :], psc[:])
                    pos += seg
        def seg_scatter(acc, col0, ctb, dst_sb, key):
            # one call per offset segment: dsts unique within a segment, so no
            # DMA read-modify-write races; calls serialize via WAW tracking
            pos = 0
            for o in range(8):
                seg = sizes["segs"][key][o]
                nc.gpsimd.dma_scatter_add(
                    acc[:, col0 : col0 + 16],
                    ctb[:, pos // 128 : (pos + seg) // 128, :],
                    dst_sb[:, pos // 16 : (pos + seg) // 16],
                    seg,
                    seg,
                    16,
                    elem_step=64,
                )
                pos += seg

        seg_scatter(acc1, 0, contrib1, dst1_sb, "p1")
        seg_scatter(acc1, 16, contrib2, dst2_sb, "p2")

        # ================= lrelu + hold + masked stats =================
        def stats_and_hold(acc, held, statx, statq, nck):
            with (
                tc.tile_pool(name="rb", bufs=3) as rbp,
                tc.tile_pool(name="sq", bufs=2) as sqp,
                tc.tile_pool(name="pst", bufs=2, space=bass.MemorySpace.PSUM) as pst,
            ):
                for c in range(nck):
                    rb = rbp.tile([128, 4, 64], f32)
                    nc.sync.dma_start(rb[:], dram3(acc, c * CHUNK, 64, 4, 64))
                    hslice = held[:, 4 * c : 4 * c + 4, :]
                    # lrelu(x) = max(x, 0.01*x)
                    tl = sqp.tile([128, 4, 32], f32)
                    nc.scalar.mul(tl[:], rb[:, :, 0:32], NEG)
                    nc.vector.tensor_tensor(hslice, rb[:, :, 0:32], tl[:], ALU.max)
                    if c < OWN_CHUNKS:
                        full = c < OWN_CHUNKS - 1
                        sq = sqp.tile([128, 4, 32], bf16)
                        nc.vector.tensor_tensor(sq[:], hslice, hslice, ALU.mult)
                        for stt, dat in ((statx, hslice), (statq, sq[:])):
                            pss = pst.tile([1, 128], f32)
                            if full:
                                nc.tensor.matmul(pss[:], ones_sb[:], dat)
                            else:
                                nc.tensor.matmul(
                                    pss[:, 0:96], ones_sb[:], dat[:, 0:3, :]
                                )
                                nc.tensor.matmul(
                                    pss[:, 96:128], maskL_sb[:], dat[:, 3, :]
                                )
                            nc.vector.tensor_tensor(
                                stt[:], stt[:], pss[:], ALU.add
                            )

        stats_and_hold(acc1, held1, stat1x, stat1q, n_chunks)

        def finish_stats(statx, statq, ccin, ccout, gb_sb, ab_sb):
            for stt, lo in ((statx, 0), (statq, 32)):
                dstc = scratch[:, lo : lo + 32]
                nc.vector.tensor_tensor(dstc, stt[:, 0:32], stt[:, 32:64], ALU.add)
                nc.vector.tensor_tensor(dstc, dstc, stt[:, 64:96], ALU.add)
                nc.vector.tensor_tensor(dstc, dstc, stt[:, 96:128], ALU.add)
            nc.sync.dma_start(ccin[:], scratch[:])
            nc.gpsimd.collective_compute(
                "AllReduce",
                ALU.add,
                replica_groups=[list(range(NCORES))],
                ins=[ccin.ap().opt()],
                outs=[ccout.ap().opt()],
            )
            with tc.tile_pool(name="st", bufs=1) as stp:
                mom = stp.tile([1, 64], f32)  # [mean | E x2]
                nc.sync.dma_start(mom[:], ccout[:])
                nc.scalar.mul(mom[:], mom[:], 1.0 / N_VOX)
                var = stp.tile([1, 32], f32)
                nc.scalar.activation(var[:], mom[:, 0:32], AF.Square, bias=zb[0:1, :])
                nc.vector.tensor_tensor(var[:], mom[:, 32:64], var[:], ALU.subtract)
                nc.vector.tensor_scalar_add(var[:], var[:], EPS)
                inv = stp.tile([1, 32], f32)
                nc.vector.reciprocal(inv[:], var[:])
                nc.scalar.activation(inv[:], inv[:], AF.Sqrt, bias=zb[0:1, :])
                ab = stp.tile([1, 64], f32)
                nc.vector.tensor_tensor(ab[:, 0:32], inv[:], gb_sb[:, 0:32], ALU.mult)
                nc.vector.tensor_tensor(ab[:, 32:64], mom[:, 0:32], ab[:, 0:32], ALU.mult)
                nc.vector.tensor_tensor(
                    ab[:, 32:64], gb_sb[:, 32:64], ab[:, 32:64], ALU.subtract
                )
                onebf = stp.tile([1, 128], f32)
                nc.vector.memset(onebf[:], 1.0)
                with tc.tile_pool(name="pab", bufs=1, space=bass.MemorySpace.PSUM) as pabp:
                    pab = pabp.tile([128, 64], f32)
                    nc.tensor.matmul(pab[:], onebf[:], ab[:])
                    nc.vector.tensor_copy(ab_sb[:], pab[:])

        finish_stats(stat1x, stat1q, ccin1, ccout1, gb1_sb, ab1_sb)

        # ---- apply BN1, emit row-major bf16 + transposed bf16 (owned) ----
        A1b = bcast4(ab1_sb[:, 0:32], 4)
        B1b = bcast4(ab1_sb[:, 32:64], 4)
        with tc.tile_pool(name="pT", bufs=2, space=bass.MemorySpace.PSUM) as pTp:
            for c in range(n_chunks):
                hslice = held1[:, 4 * c : 4 * c + 4, :]
                w = wn[c % 2]
                y = w[:, :, 0:32]
                nc.vector.tensor_tensor(y, hslice, A1b, ALU.mult)
                nc.vector.tensor_tensor(y, y, B1b, ALU.add)
                nc.scalar.dma_start(dram3(s1r1n, c * CHUNK, 128, 4, 128), w[:])
                if c < OWN_CHUNKS:
                    psT = pTp.tile([32, 4, 128], bf16)
                    for bk in range(4):
                        nc.tensor.transpose(psT[:, bk, :], y[:, bk, :], ident_sb[:])
                    nc.vector.tensor_copy(
                        AP(s1r1T, c * CHUNK, [[OWNpad, 32], [128, 4], [1, 128]]),
                        psT[:],
                    )

        # ================= stage 2 =================
        # SWDGE ring holds ~1024 descriptors; chunk the gather into 512-desc calls
        for g0 in range(0, P34, 512):
            nc.gpsimd.dma_gather(
                gath[:, :, g0 : g0 + 512],
                s1r1n[:],
                gsrc_sb[:, g0 // 16 : (g0 + 512) // 16],
                512,
                512,
                128,
                transpose=True,
            )
        with tc.tile_pool(name="ps3", bufs=2, space=bass.MemorySpace.PSUM) as ps3:
            for c in range(OWN_CHUNKS):
                ps = ps3.tile([128, 4, 32], f32)
                for bk in range(4):
                    nc.tensor.matmul(
                        ps[:, bk, :],
                        s1r1T[:, c * CHUNK + bk * 128 : c * CHUNK + (bk + 1) * 128],
                        Wc2_sb[:],
                    )
                w = wa[c % 2]
                nc.scalar.copy(w[:, :, 0:32], ps[:])
                nc.scalar.dma_start(dram3(acc2, c * CHUNK, 64, 4, 64), w[:])
        with tc.tile_pool(name="ps4", bufs=2, space=bass.MemorySpace.PSUM) as ps4:
            for (base, wo_sb, ctb, key) in (
                (0, W12o_sb, contrib3, "p3"),
                (P3, W3o_sb, contrib4, "p4"),
            ):
                pos = 0
                for o in range(8):
                    seg = sizes["segs"][key][o]
                    for s0 in range(pos, pos + seg, 128):
                        psc = ps4.tile([128, 16], f32)
                        nc.tensor.matmul(
                            psc[:],
                            gath[0:32, 0, base + s0 : base + s0 + 128],
                            wo_sb[:, o, :],
                        )
                        nc.vector.tensor_copy(ctb[:, s0 // 128, :], psc[:])
                    pos += seg
        seg_scatter(acc2, 0, contrib3, dst3_sb, "p3")
        seg_scatter(acc2, 16, contrib4, dst4_sb, "p4")

        # ================= stage 2.5 =================
        stats_and_hold(acc2, held2, stat2x, stat2q, OWN_CHUNKS)
        finish_stats(stat2x, stat2q, ccin2, ccout2, gb2_sb, ab2_sb)

        A2s = bcast4(ab2_sb[:, 0:16])
        A2r = bcast4(ab2_sb[:, 16:32])
        nc.vector.tensor_tensor(B2sum[:], ab2_sb[:, 32:48], ab2_sb[:, 48:64], ALU.add)
        B2b = bcast4(B2sum[:])
        with tc.tile_pool(name="ob", bufs=3) as obp:
            for c in range(OWN_CHUNKS):
                hs = held2[:, 4 * c : 4 * c + 4, 0:16]
                hr = held2[:, 4 * c : 4 * c + 4, 16:32]
                ob = obp.tile([128, 4, 16], f32)
                t2 = obp.tile([128, 4, 16], f32)
                nc.vector.tensor_tensor(ob[:], hs, A2s, ALU.mult)
                nc.vector.tensor_tensor(t2[:], hr, A2r, ALU.mult)
                nc.vector.tensor_tensor(ob[:], ob[:], t2[:], ALU.add)
                nc.vector.tensor_tensor(ob[:], ob[:], B2b, ALU.add)
                base = c * CHUNK
                if base + CHUNK <= M_OWN:
                    nc.scalar.dma_start(dram3(out_ext, base, C_OUT, 4, C_OUT), ob[:])
                else:
                    nfull = (M_OWN - base) // 128
                    rem = M_OWN - base - nfull * 128
                    if nfull:
                        nc.scalar.dma_start(
                            dram3(out_ext, base, C_OUT, nfull, C_OUT),
                            ob[:, 0:nfull, :],
                        )
                    if rem:
                        dst = AP(
                            out_ext,
                            (base + nfull * 128) * C_OUT,
                            [[C_OUT, rem], [1, C_OUT]],
                        )
                        nc.scalar.dma_start(dst, ob[0:rem, nfull, :])

    nc.compile()
    return nc


def _install_ntff_shim():
    """bass_utils trace=True imports antenv.axon_hooks, absent in this image.
    Provide it, backed by the ctypes NTFF profiler in trn_agent_boot."""
    import sys as _sys
    import types as _types

    try:
        import antenv.axon_hooks  # noqa: F401
        return
    except ImportError:
        pass
    hook = None
    try:
        from trn_agent_boot.trn_boot import _ntff_profile_via_ctypes

        hook = _ntff_profile_via_ctypes("/opt/axon/libaxon_pjrt.so")
    except Exception:
        hook = None
    mod = _types.ModuleType("antenv.axon_hooks")
    mod.get_axon_ntff_profile_hook = lambda: hook
    mod.set_axon_ntff_profile_hook = lambda h: None
    _sys.modules["antenv.axon_hooks"] = mod


def kernel(**inputs):
    global LAST_RESULT
    feats = np.asarray(inputs["feats"], np.float32)
    coords = np.asarray(inputs["coords"], np.int32)

    cores, sizes, in_maps = _host_prep(feats, coords)
    nc = _build_graph(
        sizes,
        *[
            np.asarray(inputs[k], np.float32)
            for k in ("W1", "g0", "b0", "W1_2", "g0_2", "b0_2", "W2", "g1", "b1", "W3", "g2", "b2")
        ],
    )

    _install_ntff_shim()
    from concourse.bass_utils import run_bass_kernel_spmd

    res = run_bass_kernel_spmd(
        nc,
        in_maps,
        core_ids=list(range(NCORES)),
        trace=bool(os.environ.get("BASS_TRACE")),
    )
    LAST_RESULT = res

    full = np.empty((N_VOX, C_OUT), np.float32)
    for k in range(NCORES):
        full[cores[k]["own_g"]] = np.asarray(res.results[k]["out"], np.float32)
    return full
